# revision 18
# baseline (speedup 1.0000x reference)
"""Trainium2 Bass kernel for nn_MlpwithSOMModule (pairwise-concat MLP + max/mask/sum).

Reference computation (B=8, C=4, T=128, D=64, H=128, G=B*C=32):
  entity  = input[:,:,1] -> [G,T,D];  context = input[:,:,0] -> [G,T,D]
  mask    = (context[:,:,0] != 0)                         [G,T]
  x[g,i,j] = concat(context[g,i], entity[g,j])            [G,T,T,2D]
  for l in 0..5: x = tanh(x @ Ws[l] + bs[l])
  score  = (x @ W_out + b_out)[...,0]                     [G,T,T]
  out[g] = sum_i( max_j(score[g,i,j]) * mask[g,i] )       [G]

Sharding: data-parallel over G across 8 cores (4 groups/core); weights
replicated.  On-chip layout is feature-major ([128 features, pairs]) so every
MLP layer is one stationary-weight matmul.  Layer 0 uses the concat split:
  x0 = ctx_i @ W0[:D] + ent_j @ W0[D:]  ->  A[:,i] + Bb[:,j]
with A, Bb computed once per group as [128,128] matrices; the z tiles are
built by a broadcast-AP add on the (otherwise idle) GpSimd engine.

The throughput limit is tanh: 6 activation layers x 64Ki pairs x 128 feats
per core, with BOTH activation engines at ~1 elem/cycle/lane.  Layers 3-5
run exact tanh on the Scalar (ACT) engine; layers 0-2 run on the Vector
engine as a SINGLE fused custom-DVE op per tile -- a degree-5 odd
polynomial with clip whose leading coefficient is folded into prescaled
weights (z' = alpha*z), so the body is x*(x^2(x^2+C0)+C1) clipped to
[-1,1]: 7 ALU stages, 1 elem/cycle.  The deg-5 coefficients are
least-squares-tuned per layer against the actual z distribution
(bias-penalized): the systematic part of the approximation error is what
survives the max/sum reduction, so this keeps end-to-end error ~9e-3.
Some layer-0 tiles run exact tanh on ACT instead (scale=1/alpha undoes the
prescale) to equalize ACT and DVE busy time.

The MLP is software-pipelined one stage per position.  Layers 1-2 advance
in 512-pair units ([H,512] one-bank PSUM tiles); layers 3-5 and layer 0
advance in 1024-pair unit-PAIRS ([H,1024] two-bank tiles) because the
~180ns ACT instruction overhead (cayman read-write bubble) amortizes much
better at 1024 columns.  PSUM budget: 3x[H,512] + 2x[H,1024] + 1 score
bank = exactly 8 banks; pair tiles are produced and consumed within ~1
position.  PE matmuls trail the activations and self-throttle on PSUM
recycling.
"""

import numpy as np
import ml_dtypes

import concourse.bacc as bacc
import concourse.mybir as mybir
import concourse.tile as tile
from concourse.bass_utils import run_bass_kernel_spmd

B, C, T, D = 8, 4, 128, 64
H = 2 * D          # 128
G = B * C          # 32 groups
N_CORES = 8
G_LOC = G // N_CORES   # 4 groups per core
NJ_BLOCK = 16          # j's per z-build block
BLOCK = NJ_BLOCK * T   # 2048 pairs per z block
N_BLOCKS = (G_LOC * T) // NJ_BLOCK   # 32 z blocks per core
UNIT = 512             # pairs per pipeline unit (4 j's)
N_UNITS = G_LOC * T * T // UNIT      # 128 units per core
N_PAIRS = N_UNITS // 2               # 64 unit-pairs
U_PER_BLOCK = BLOCK // UNIT          # 4
U_PER_GROUP = T * T // UNIT          # 32
N_ACT_T1 = 19          # tanh1 unit-pairs on ACT (exact tanh) for balance
ACT_T1_PAIRS = frozenset(round(i * N_PAIRS / N_ACT_T1) for i in range(N_ACT_T1))
# edge specialization: during pipeline fill the DVE is the serial bottleneck
# (layers 0-2), so the first units' L1/L2 run exact tanh on the idle ACT
# (unscaled weights); during drain only ACT work remains, so the last pairs'
# L3-5 run as DVE polys (alpha-scaled weights).  Both disabled when biases
# are nonzero (the poly bias fold isn't plumbed for these paths).
EARLY_ACT_UNITS = 4
TAIL_P5_PAIRS = frozenset({61, 62, 63})

F32 = mybir.dt.float32
BF16 = mybir.dt.bfloat16
AF = mybir.ActivationFunctionType
ALU = mybir.AluOpType
AX = mybir.AxisListType

# Per-layer deg-5 odd tanh fits: tanh(z) ~= clip(c*z*(z^4 + u z^2 + v), -1, 1),
# least-squares on the empirical z distribution of that layer with a bias
# penalty.  The leading coefficient folds into the weights: alpha = c^(1/5),
# x = alpha*z, p = x*(x^4 + u' x^2 + v') with u' = u*alpha^2, v' = v*alpha^4.
P5 = {
    0: (-9.877681842925131, 46.69095161365766, 0.020506639556395076),
    1: (-9.657263893294322, 44.24494502377194, 0.02187289699293519),
    2: (-9.637965965161936, 43.8571751851745, 0.022186182800311267),
    3: (-9.620118349674474, 43.49903927798162, 0.022483764232557085),
    4: (-9.616913856312081, 43.43417015920825, 0.02253864408388847),
    5: (-9.611545692545945, 43.325487083958954, 0.022631196446351306),
}
ALPHA = {l: P5[l][2] ** 0.2 for l in P5}
P5S = {l: (P5[l][0] * ALPHA[l] ** 2, P5[l][1] * ALPHA[l] ** 4) for l in P5}

_cached_nc = {}


def _register_p5_op():
    """Register the fused deg-5 tanh custom DVE op (idempotent).
      y = clip(((sq(x) + C0) * sq(x) + C1) * x, -1, 1)   [7 ALU stages]
    C0/C1 are runtime scalars, so one op serves all layers."""
    import concourse.dve_ops as DO
    from concourse.dve_spec import Spec, Src0, C0, C1, Zero, One, \
        sq, maxx, minn, lower
    from concourse.dve_uop import DveOpSpec
    from concourse.dve_table_gen import dve_ver_for
    from concourse.dve_ops import has_src1

    if "TANH_P5" in DO._SUB_OPCODE_FOR_NAME:
        return DO._P5_OP

    t = sq(Src0)
    body = minn(maxx(((t + C0) * t + C1) * Src0, Zero - One), One)

    ver = dve_ver_for("TRN2")
    spec = Spec(body=body)
    row = DO._CUSTOM_DVE_ROW_BASE + len(DO.OPS)
    tmp = DveOpSpec(name="TANH_P5", opcode=row, uops=lower(spec, ver=ver),
                    rd1_en=has_src1(spec))
    op = DO.DveOp("TANH_P5", spec, subdim=False, uops_sha={ver: tmp.sha(ver)})
    DO.OPS.append(op)
    DO._SUB_OPCODE_FOR_NAME["TANH_P5"] = row
    DO.CUSTOM_DVE_SPECS["TANH_P5"] = spec
    DO._P5_OP = op
    return op


def _build_program(bias_zero):
    op5 = _register_p5_op()
    nc = bacc.Bacc("TRN2", target_bir_lowering=False, debug=False,
                   num_devices=N_CORES)

    aT_d = nc.dram_tensor("aT", [G_LOC, H, T], BF16, kind="ExternalInput")
    bbT_d = nc.dram_tensor("bbT", [G_LOC, H, T], BF16, kind="ExternalInput")
    ctx0_d = nc.dram_tensor("ctx0", [G_LOC, T, 1], F32, kind="ExternalInput")
    ws_d = nc.dram_tensor("Ws", [6, H, H], BF16, kind="ExternalInput")
    wsx_d = nc.dram_tensor("Wsx", [5, H, H], BF16, kind="ExternalInput")
    bsT_d = nc.dram_tensor("bsT", [H, 6], F32, kind="ExternalInput")
    bsrow_d = nc.dram_tensor("bsrow", [1, 6 * H], BF16, kind="ExternalInput")
    wout_d = nc.dram_tensor("wout", [H, 1], BF16, kind="ExternalInput")
    bout_d = nc.dram_tensor("bout", [T, 1], F32, kind="ExternalInput")
    out_d = nc.dram_tensor("out", [1, G_LOC], F32, kind="ExternalOutput")

    with tile.TileContext(nc) as tc:
        with (
            tc.tile_pool(name="consts", bufs=1) as consts,
            tc.tile_pool(name="zpool", bufs=4) as zpool,
            tc.tile_pool(name="hpool", bufs=10) as hpool,
            tc.tile_pool(name="hppool", bufs=10) as hppool,
            tc.tile_pool(name="small", bufs=4) as small,
            tc.tile_pool(name="mmps", bufs=3, space="PSUM") as mmps,
            tc.tile_pool(name="prps", bufs=2, space="PSUM") as prps,
            tc.tile_pool(name="scps", bufs=1, space="PSUM") as scps,
        ):
            # dummy activation first: pulls the tanh ACT_TABLE_LOAD (~1.3us)
            # off the critical path, overlapping it with setup DMAs
            scratch_sb = consts.tile([1, 1], F32)
            scratch2_sb = consts.tile([1, 1], F32)
            nc.gpsimd.memset(scratch_sb[:], 0.0)
            nc.scalar.activation(scratch2_sb[:], scratch_sb[:], AF.Tanh)

            ws_sb = consts.tile([H, 11 * H], BF16)
            bsT_sb = consts.tile([H, 6], F32)
            wout_sb = consts.tile([H, 1], BF16)
            bout_sb = consts.tile([T, 1], F32)
            ones_sb = consts.tile([T, 1], F32)
            res_sb = consts.tile([1, G_LOC], F32)
            bsrow_sb = consts.tile([1, 6 * H], BF16)
            nc.sync.dma_start(bsrow_sb[:], bsrow_d[:])
            ones512_sb = consts.tile([1, UNIT], BF16)
            nc.vector.memset(ones512_sb[:], 1.0)

            # Per-group setup: A/Bb first-layer matrices (alpha0-prescaled via
            # the host-side weight scaling), mask sources, running max.
            a_sbs, bb_sbs, ctx0_sbs, bmax_sbs = [None] * G_LOC, \
                [None] * G_LOC, [None] * G_LOC, [None] * G_LOC

            def setup_group(g):
                # A = (ctx @ a0*W0_top).T and Bb = (ent @ a0*W0_bot).T + a0*b0
                # are precomputed on the host (0.1% of the FLOPs)
                a_sb = consts.tile([H, T], BF16, tag=f"a{g}")
                bb_sb = consts.tile([H, T], BF16, tag=f"bb{g}")
                ctx0_sb = consts.tile([T, 1], F32, tag=f"ctx0_{g}")
                nc.sync.dma_start(a_sb[:], aT_d[g])
                nc.sync.dma_start(bb_sb[:], bbT_d[g])
                nc.sync.dma_start(ctx0_sb[:], ctx0_d[g])
                bmax_sb = consts.tile([T, N_BLOCKS // G_LOC], F32,
                                      tag=f"bmax{g}")
                a_sbs[g], bb_sbs[g] = a_sb, bb_sb
                ctx0_sbs[g], bmax_sbs[g] = ctx0_sb, bmax_sb

            from concourse.bass import broadcast_tensor_aps

            def _z_add(engine, z_sb, g, j0, nj, c0):
                a_ap = a_sbs[g][:].rearrange("p (o i) -> p o i", o=1)
                b_ap = bb_sbs[g][:, j0:j0 + nj].rearrange("p (j o) -> p j o",
                                                          o=1)
                a_b, b_b = broadcast_tensor_aps(a_ap, b_ap)
                z_ap = z_sb[:, c0:c0 + nj * T].rearrange("p (j i) -> p j i",
                                                         j=nj)
                engine.tensor_tensor(z_ap, a_b, b_b, op=ALU.add)

            def build_z(b, fill=False):
                """Layer-0 z for block b (16 j's): z[:, jl*T+i] = A[:,i]+Bb[:,j].
                Broadcast-AP adds on the GpSimd engine; during pipeline fill
                the first piece goes to the (still idle) DVE and the rest is
                split so tanh1 can start before the whole block is done."""
                g = b // (N_BLOCKS // G_LOC)
                c = b % (N_BLOCKS // G_LOC)
                z_sb = zpool.tile([H, BLOCK], BF16, tag="z")
                j0 = c * NJ_BLOCK
                if fill:
                    _z_add(nc.vector, z_sb, g, j0, 4, 0)
                    for s in range(1, 4):
                        _z_add(nc.gpsimd, z_sb, g, j0 + 4 * s, 4, 4 * s * T)
                else:
                    _z_add(nc.gpsimd, z_sb, g, j0, NJ_BLOCK, 0)
                return z_sb

            z_tiles = {}
            h_cur = {}
            hp_cur = {}
            sc_tiles = {}

            def stage_t1_pair(e):
                """Unit-pair e's layer-0 activation on the z half-block
                [H,1024]: deg-5 DVE poly, or exact ACT tanh (scale undoes the
                alpha0 prescale) on some pairs to balance the engines."""
                b, s = e // 2, e % 2
                hp_sb = hppool.tile([H, 2 * UNIT], BF16, tag="hp")
                zsl = z_tiles[b][:, s * 2 * UNIT:(s + 1) * 2 * UNIT]
                if e in ACT_T1_PAIRS:
                    nc.scalar.activation(hp_sb[:], zsl, AF.Tanh,
                                         scale=1.0 / ALPHA[0])
                else:
                    nc.vector._custom_dve(op5, out=hp_sb[:], in0=zsl,
                                          s0=P5S[0][0], s1=P5S[0][1])
                hp_cur[e] = hp_sb
                if s == 1:
                    del z_tiles[b]

            def stage_mm(u, l):
                # layers 1-2: one 512-col matmul into a one-bank PSUM tile
                if l == 1:
                    ht = hp_cur[u // 2][:, (u % 2) * UNIT:(u % 2 + 1) * UNIT]
                else:
                    ht = h_cur[u][:]
                early = bias_zero and u < EARLY_ACT_UNITS
                slot = (5 + l) if early else l
                ps = mmps.tile([H, UNIT], F32, tag="mm")
                need_bias_mm = not bias_zero
                if need_bias_mm:
                    # fold the (prescaled) bias in on the PE -- the DVE poly
                    # has no bias slot
                    nc.tensor.matmul(ps[:], bsrow_sb[0:1, l * H:(l + 1) * H],
                                     ones512_sb[:], start=True, stop=False)
                nc.tensor.matmul(ps[:], ws_sb[:, slot * H:(slot + 1) * H], ht,
                                 start=not need_bias_mm, stop=True)
                h_cur[u] = ps

            def stage_act(u, l):
                # layers 1-2: deg-5 poly on the DVE; during fill the first
                # units go exact on the (otherwise idle) ACT instead
                ps = h_cur[u]
                h2_sb = hpool.tile([H, UNIT], BF16, tag="h")
                if bias_zero and u < EARLY_ACT_UNITS:
                    nc.scalar.activation(h2_sb[:], ps[:], AF.Tanh)
                else:
                    nc.vector._custom_dve(op5, out=h2_sb[:], in0=ps[:],
                                          s0=P5S[l][0], s1=P5S[l][1])
                h_cur[u] = h2_sb

            def stage_pair(e, l):
                # layers 3-5: two 512-col matmuls into one [H,1024] two-bank
                # PSUM tile, then ONE exact-tanh ACT op over the pair (the
                # ~180ns ACT instruction overhead amortizes at 1024 cols).
                # During drain only ACT work remains, so the last pairs run
                # the deg-5 poly on the (otherwise idle) DVE instead.
                tail = bias_zero and e in TAIL_P5_PAIRS
                slot = (l + 5) if tail else l
                ps = prps.tile([H, 2 * UNIT], F32, tag="pr")
                for s in range(2):
                    u = 2 * e + s
                    if l == 3:
                        ht = h_cur.pop(u)[:]
                    else:
                        ht = hp_cur[e][:, s * UNIT:(s + 1) * UNIT]
                    nc.tensor.matmul(ps[:, s * UNIT:(s + 1) * UNIT],
                                     ws_sb[:, slot * H:(slot + 1) * H], ht,
                                     start=True, stop=True)
                hp_sb = hppool.tile([H, 2 * UNIT], BF16, tag="hp")
                if tail:
                    nc.vector._custom_dve(op5, out=hp_sb[:], in0=ps[:],
                                          s0=P5S[l][0], s1=P5S[l][1])
                else:
                    nc.scalar.activation(hp_sb[:], ps[:], AF.Tanh,
                                         bias=bsT_sb[:, l:l + 1])
                hp_cur[e] = hp_sb

            def stage_score(u):
                # final layer, transposed: per j, score col [T(i), 1] via
                # stationary h-block x moving W_out
                b, s = u // U_PER_BLOCK, u % U_PER_BLOCK
                if s == 0:
                    sc_new = scps.tile([H, UNIT], F32, tag="sc")
                    sc_tiles[b] = sc_new
                sc_ps = sc_tiles[b]
                e, half = u // 2, u % 2
                ht = hp_cur[e]
                for jl in range(4):
                    o = half * UNIT + jl * T
                    nc.tensor.matmul(
                        sc_ps[:, s * 4 + jl:s * 4 + jl + 1],
                        ht[:, o:o + T],
                        wout_sb[:], start=True, stop=True)
                if half == 1:
                    del hp_cur[e]

            def stage_reduce(u):
                # max over the block's 16 j's into the group's per-block max
                # column (independent writes -- no serial running-max chain)
                b = u // U_PER_BLOCK
                nb = N_BLOCKS // G_LOC
                g, bl = b // nb, b % nb
                nc.vector.tensor_reduce(bmax_sbs[g][:, bl:bl + 1],
                                        sc_tiles.pop(b)[:, 0:NJ_BLOCK],
                                        axis=AX.X, op=ALU.max)

            def finalize_group(g):
                # mask = (ctx[:,0] != 0); out = sum_i(mask*(rmax+b_out))
                mask_sb = small.tile([T, 1], F32, tag="mask")
                nc.vector.tensor_scalar(mask_sb[:], ctx0_sbs[g][:], 0.0, None,
                                        op0=ALU.not_equal)
                rmax_sb = small.tile([T, 1], F32, tag="rmax")
                nc.vector.tensor_reduce(rmax_sb[:], bmax_sbs[g][:],
                                        axis=AX.X, op=ALU.max)
                rb_sb = small.tile([T, 1], F32, tag="rb")
                nc.vector.tensor_scalar_add(rb_sb[:], rmax_sb[:],
                                            bout_sb[:, 0:1])
                mm_sb = small.tile([T, 1], F32, tag="mmul")
                nc.vector.tensor_mul(mm_sb[:], rb_sb[:], mask_sb[:])
                # partition-axis sum via ones-matmul: [1,1] = mm.T @ ones
                sum_ps = scps.tile([H, UNIT], F32, tag="sc")
                nc.tensor.matmul(sum_ps[0:1, 0:1], mm_sb[:], ones_sb[:],
                                 start=True, stop=True)
                nc.vector.tensor_copy(res_sb[0:1, g:g + 1], sum_ps[0:1, 0:1])

            # group 0 first (the z(0) -> tanh1 chain gates pipeline fill),
            # then the weight DMAs (layer 1 isn't needed until position ~5)
            setup_group(0)
            nc.sync.dma_start(ws_sb[:, H:2 * H], ws_d[1])
            nc.sync.dma_start(bsT_sb[:], bsT_d[:])
            for l in range(2, 6):
                nc.sync.dma_start(ws_sb[:, l * H:(l + 1) * H], ws_d[l])
            for k in range(5):
                nc.sync.dma_start(ws_sb[:, (6 + k) * H:(7 + k) * H], wsx_d[k])
            nc.sync.dma_start(wout_sb[:], wout_d[:])
            nc.sync.dma_start(bout_sb[:], bout_d[:])
            nc.vector.memset(ones_sb[:], 1.0)

            # Software pipeline, one stage per position.  Layers 1-2 advance
            # per 512-pair unit u; layer 0 and layers 3-5 advance per
            # unit-pair e (positions 2e+d).  Stage delays interleave the two
            # granularities so each engine sees a near-constant load; the
            # emission order within a position respects same-position
            # producer/consumer hand-offs (L2act(2e+1) before the L3 pair
            # matmuls; the block reduce before the next block's score, which
            # recycles the single score PSUM buffer).
            D_T1 = 3          # tanh1 pair at 2e+3 (odd positions)
            D_MM1, D_ACT1 = 5, 6
            D_MM2, D_ACT2 = 7, 8
            D_L3 = 10         # pair mms+act at 2e+10 (even)
            D_L4 = 13         # 2e+13 (odd)
            D_L5 = 16         # 2e+16 (even)
            D_SC = 17         # score per unit at u+17
            D_RED = 18        # block reduce at (4b+3)+18
            NPOS = N_UNITS + D_RED + 1

            def live(u):
                return 0 <= u < N_UNITS

            for p in range(NPOS):
                # group setup trickles in ahead of its first unit
                for g in range(1, G_LOC):
                    if p == g * U_PER_GROUP - 10:
                        setup_group(g)
                # GpSimd: z block every 4 positions, 3 ahead of its tanh1
                if p % U_PER_BLOCK == 0 and p // U_PER_BLOCK < N_BLOCKS:
                    b = p // U_PER_BLOCK
                    z_new = build_z(b, fill=(b == 0))
                    z_tiles[b] = z_new
                if live(p - D_RED) and (p - D_RED) % U_PER_BLOCK == 3:
                    stage_reduce(p - D_RED)
                # PE, oldest first
                if live(p - D_SC):
                    stage_score(p - D_SC)
                if (p - D_L5) % 2 == 0 and live(p - D_L5):
                    stage_pair((p - D_L5) // 2, 5)
                if (p - D_L4) % 2 == 0 and live(p - D_L4):
                    stage_pair((p - D_L4) // 2, 4)
                # DVE acts for layers 1-2 (L2act(2e+1) feeds this position's
                # L3 pair matmuls)
                if live(p - D_ACT2):
                    stage_act(p - D_ACT2, 2)
                if live(p - D_ACT1):
                    stage_act(p - D_ACT1, 1)
                if (p - D_L3) % 2 == 0 and live(p - D_L3):
                    stage_pair((p - D_L3) // 2, 3)
                if live(p - D_MM2):
                    stage_mm(p - D_MM2, 2)
                if live(p - D_MM1):
                    stage_mm(p - D_MM1, 1)
                if (p - D_T1) % 2 == 0 and live(p - D_T1):
                    stage_t1_pair((p - D_T1) // 2)
                # per-group finalize as soon as its last reduce is in
                for g in range(G_LOC - 1):
                    if p == (g + 1) * U_PER_GROUP - 1 + D_RED + 1:
                        finalize_group(g)

            finalize_group(G_LOC - 1)

            nc.sync.dma_start(out_d[:], res_sb[:])

    nc.compile()
    return nc


def _get_nc(bias_zero):
    if bias_zero not in _cached_nc:
        _cached_nc[bias_zero] = _build_program(bias_zero)
    return _cached_nc[bias_zero]


def _bf16(a):
    return np.ascontiguousarray(a.astype(ml_dtypes.bfloat16))


def _prep_in_maps(input, Ws, bs, W_out, b_out):
    input = np.ascontiguousarray(np.asarray(input, dtype=np.float32))
    Ws = np.asarray(Ws, dtype=np.float32)
    bs = np.asarray(bs, dtype=np.float32)
    W_out = np.asarray(W_out, dtype=np.float32)
    b_out = np.asarray(b_out, dtype=np.float32)

    ctx = input[:, :, 0].reshape(G, T, D)
    ent = input[:, :, 1].reshape(G, T, D)
    ctx0 = np.ascontiguousarray(ctx[:, :, 0]).reshape(G, T, 1)  # fp32
    # prescale: alpha_l folds the deg-5 leading coefficient into the weights
    Ws_mod = Ws.copy()
    for l in range(3):
        Ws_mod[l] = ALPHA[l] * Ws[l]
    ws_bf = _bf16(Ws_mod)
    wsx = _bf16(np.stack([Ws[1], Ws[2]] +
                         [ALPHA[l] * Ws[l] for l in (3, 4, 5)]))
    # layer-0 split, computed host-side with the same bf16 rounding the PE
    # would apply: A = (ctx @ a0*W0_top).T, Bb = (ent @ a0*W0_bot).T + a0*b0
    w0t_f = _bf16(ALPHA[0] * Ws[0][0:D]).astype(np.float32)
    w0b_f = _bf16(ALPHA[0] * Ws[0][D:H]).astype(np.float32)
    ctx_f = _bf16(ctx).astype(np.float32)
    ent_f = _bf16(ent).astype(np.float32)
    aT = _bf16((ctx_f @ w0t_f).transpose(0, 2, 1))        # [G, H, T]
    bbT = _bf16((ent_f @ w0b_f).transpose(0, 2, 1)
                + (ALPHA[0] * bs[0]).reshape(1, H, 1))
    bsT = np.ascontiguousarray(bs.T).copy()               # [H, 6]
    bsT[:, 0] *= ALPHA[0]                                 # folded into Bb
    bs_row = bs.copy()
    bs_row[1] *= ALPHA[1]
    bs_row[2] *= ALPHA[2]
    bsrow = _bf16(bs_row.reshape(1, 6 * H))
    wout = _bf16(W_out)
    bout = np.broadcast_to(b_out.reshape(1, 1), (T, 1)).copy()

    in_maps = []
    for k in range(N_CORES):
        sl = slice(k * G_LOC, (k + 1) * G_LOC)
        in_maps.append({
            "aT": np.ascontiguousarray(aT[sl]),
            "bbT": np.ascontiguousarray(bbT[sl]),
            "ctx0": np.ascontiguousarray(ctx0[sl]),
            "Ws": ws_bf,
            "Wsx": wsx,
            "bsT": bsT,
            "bsrow": bsrow,
            "wout": wout,
            "bout": bout,
        })
    return in_maps


def run_traced(trace=False, **inputs):
    """Returns (output [G], exec_time_ns or None)."""
    nc = _get_nc(bias_zero=bool(np.all(np.asarray(inputs["bs"]) == 0)))
    in_maps = _prep_in_maps(**inputs)
    res = run_bass_kernel_spmd(nc, in_maps, list(range(N_CORES)), trace=trace)
    out = np.concatenate([res.results[k]["out"].reshape(G_LOC)
                          for k in range(N_CORES)])
    return out, res.exec_time_ns


def kernel(**inputs) -> np.ndarray:
    out, _ = run_traced(trace=False, **inputs)
    return out


# revision 19
# speedup vs baseline: 1.0089x; 1.0089x over previous
"""Trainium2 Bass kernel for nn_MlpwithSOMModule (pairwise-concat MLP + max/mask/sum).

Reference computation (B=8, C=4, T=128, D=64, H=128, G=B*C=32):
  entity  = input[:,:,1] -> [G,T,D];  context = input[:,:,0] -> [G,T,D]
  mask    = (context[:,:,0] != 0)                         [G,T]
  x[g,i,j] = concat(context[g,i], entity[g,j])            [G,T,T,2D]
  for l in 0..5: x = tanh(x @ Ws[l] + bs[l])
  score  = (x @ W_out + b_out)[...,0]                     [G,T,T]
  out[g] = sum_i( max_j(score[g,i,j]) * mask[g,i] )       [G]

Sharding: data-parallel over G across 8 cores (4 groups/core); weights
replicated.  On-chip layout is feature-major ([128 features, pairs]) so every
MLP layer is one stationary-weight matmul.  Layer 0 uses the concat split:
  x0 = ctx_i @ W0[:D] + ent_j @ W0[D:]  ->  A[:,i] + Bb[:,j]
with A, Bb computed once per group as [128,128] matrices; the z tiles are
built by a broadcast-AP add on the (otherwise idle) GpSimd engine.

The throughput limit is tanh: 6 activation layers x 64Ki pairs x 128 feats
per core, with BOTH activation engines at ~1 elem/cycle/lane.  Layers 3-5
run exact tanh on the Scalar (ACT) engine; layers 0-2 run on the Vector
engine as a SINGLE fused custom-DVE op per tile -- a degree-5 odd
polynomial with clip whose leading coefficient is folded into prescaled
weights (z' = alpha*z), so the body is x*(x^2(x^2+C0)+C1) clipped to
[-1,1]: 7 ALU stages, 1 elem/cycle.  The deg-5 coefficients are
least-squares-tuned per layer against the actual z distribution
(bias-penalized): the systematic part of the approximation error is what
survives the max/sum reduction, so this keeps end-to-end error ~9e-3.
Some layer-0 tiles run exact tanh on ACT instead (scale=1/alpha undoes the
prescale) to equalize ACT and DVE busy time.

The MLP is software-pipelined one stage per position.  Layers 1-2 advance
in 512-pair units ([H,512] one-bank PSUM tiles); layers 3-5 and layer 0
advance in 1024-pair unit-PAIRS ([H,1024] two-bank tiles) because the
~180ns ACT instruction overhead (cayman read-write bubble) amortizes much
better at 1024 columns.  PSUM budget: 3x[H,512] + 2x[H,1024] + 1 score
bank = exactly 8 banks; pair tiles are produced and consumed within ~1
position.  PE matmuls trail the activations and self-throttle on PSUM
recycling.
"""

import numpy as np
import ml_dtypes

import concourse.bacc as bacc
import concourse.mybir as mybir
import concourse.tile as tile
from concourse.bass_utils import run_bass_kernel_spmd

B, C, T, D = 8, 4, 128, 64
H = 2 * D          # 128
G = B * C          # 32 groups
N_CORES = 8
G_LOC = G // N_CORES   # 4 groups per core
NJ_BLOCK = 16          # j's per z-build block
BLOCK = NJ_BLOCK * T   # 2048 pairs per z block
N_BLOCKS = (G_LOC * T) // NJ_BLOCK   # 32 z blocks per core
UNIT = 512             # pairs per pipeline unit (4 j's)
N_UNITS = G_LOC * T * T // UNIT      # 128 units per core
N_PAIRS = N_UNITS // 2               # 64 unit-pairs
U_PER_BLOCK = BLOCK // UNIT          # 4
U_PER_GROUP = T * T // UNIT          # 32
N_ACT_T1 = 19          # tanh1 unit-pairs on ACT (exact tanh) for balance
ACT_T1_PAIRS = frozenset(round(i * N_PAIRS / N_ACT_T1) for i in range(N_ACT_T1))
# edge specialization: during pipeline fill the DVE is the serial bottleneck
# (layers 0-2), so the first units' L1/L2 run exact tanh on the idle ACT
# (unscaled weights); during drain only ACT work remains, so the last pairs'
# L3-5 run as DVE polys (alpha-scaled weights).  Both disabled when biases
# are nonzero (the poly bias fold isn't plumbed for these paths).
EARLY_ACT_UNITS = 0   # measured: the edge specializations cost ~2.5us
TAIL_P5_PAIRS = frozenset()

F32 = mybir.dt.float32
BF16 = mybir.dt.bfloat16
AF = mybir.ActivationFunctionType
ALU = mybir.AluOpType
AX = mybir.AxisListType

# Per-layer deg-5 odd tanh fits: tanh(z) ~= clip(c*z*(z^4 + u z^2 + v), -1, 1),
# least-squares on the empirical z distribution of that layer with a bias
# penalty.  The leading coefficient folds into the weights: alpha = c^(1/5),
# x = alpha*z, p = x*(x^4 + u' x^2 + v') with u' = u*alpha^2, v' = v*alpha^4.
P5 = {
    0: (-9.877681842925131, 46.69095161365766, 0.020506639556395076),
    1: (-9.657263893294322, 44.24494502377194, 0.02187289699293519),
    2: (-9.637965965161936, 43.8571751851745, 0.022186182800311267),
    3: (-9.620118349674474, 43.49903927798162, 0.022483764232557085),
    4: (-9.616913856312081, 43.43417015920825, 0.02253864408388847),
    5: (-9.611545692545945, 43.325487083958954, 0.022631196446351306),
}
ALPHA = {l: P5[l][2] ** 0.2 for l in P5}
P5S = {l: (P5[l][0] * ALPHA[l] ** 2, P5[l][1] * ALPHA[l] ** 4) for l in P5}

_cached_nc = {}


def _register_p5_op():
    """Register the fused deg-5 tanh custom DVE op (idempotent).
      y = clip(((sq(x) + C0) * sq(x) + C1) * x, -1, 1)   [7 ALU stages]
    C0/C1 are runtime scalars, so one op serves all layers."""
    import concourse.dve_ops as DO
    from concourse.dve_spec import Spec, Src0, C0, C1, Zero, One, \
        sq, maxx, minn, lower
    from concourse.dve_uop import DveOpSpec
    from concourse.dve_table_gen import dve_ver_for
    from concourse.dve_ops import has_src1

    if "TANH_P5" in DO._SUB_OPCODE_FOR_NAME:
        return DO._P5_OP

    t = sq(Src0)
    body = minn(maxx(((t + C0) * t + C1) * Src0, Zero - One), One)

    ver = dve_ver_for("TRN2")
    spec = Spec(body=body)
    row = DO._CUSTOM_DVE_ROW_BASE + len(DO.OPS)
    tmp = DveOpSpec(name="TANH_P5", opcode=row, uops=lower(spec, ver=ver),
                    rd1_en=has_src1(spec))
    op = DO.DveOp("TANH_P5", spec, subdim=False, uops_sha={ver: tmp.sha(ver)})
    DO.OPS.append(op)
    DO._SUB_OPCODE_FOR_NAME["TANH_P5"] = row
    DO.CUSTOM_DVE_SPECS["TANH_P5"] = spec
    DO._P5_OP = op
    return op


def _build_program(bias_zero):
    op5 = _register_p5_op()
    nc = bacc.Bacc("TRN2", target_bir_lowering=False, debug=False,
                   num_devices=N_CORES)

    aT_d = nc.dram_tensor("aT", [G_LOC, H, T], BF16, kind="ExternalInput")
    bbT_d = nc.dram_tensor("bbT", [G_LOC, H, T], BF16, kind="ExternalInput")
    ctx0_d = nc.dram_tensor("ctx0", [G_LOC, T, 1], F32, kind="ExternalInput")
    ws_d = nc.dram_tensor("Ws", [6, H, H], BF16, kind="ExternalInput")
    wsx_d = nc.dram_tensor("Wsx", [5, H, H], BF16, kind="ExternalInput")
    bsT_d = nc.dram_tensor("bsT", [H, 6], F32, kind="ExternalInput")
    bsrow_d = nc.dram_tensor("bsrow", [1, 6 * H], BF16, kind="ExternalInput")
    wout_d = nc.dram_tensor("wout", [H, 1], BF16, kind="ExternalInput")
    bout_d = nc.dram_tensor("bout", [T, 1], F32, kind="ExternalInput")
    out_d = nc.dram_tensor("out", [1, G_LOC], F32, kind="ExternalOutput")

    with tile.TileContext(nc) as tc:
        with (
            tc.tile_pool(name="consts", bufs=1) as consts,
            tc.tile_pool(name="zpool", bufs=4) as zpool,
            tc.tile_pool(name="hpool", bufs=10) as hpool,
            tc.tile_pool(name="hppool", bufs=10) as hppool,
            tc.tile_pool(name="small", bufs=4) as small,
            tc.tile_pool(name="mmps", bufs=3, space="PSUM") as mmps,
            tc.tile_pool(name="prps", bufs=2, space="PSUM") as prps,
            tc.tile_pool(name="scps", bufs=1, space="PSUM") as scps,
        ):
            # dummy activation first: pulls the tanh ACT_TABLE_LOAD (~1.3us)
            # off the critical path, overlapping it with setup DMAs
            scratch_sb = consts.tile([1, 1], F32)
            scratch2_sb = consts.tile([1, 1], F32)
            nc.gpsimd.memset(scratch_sb[:], 0.0)
            nc.scalar.activation(scratch2_sb[:], scratch_sb[:], AF.Tanh)

            ws_sb = consts.tile([H, 11 * H], BF16)
            bsT_sb = consts.tile([H, 6], F32)
            wout_sb = consts.tile([H, 1], BF16)
            bout_sb = consts.tile([T, 1], F32)
            ones_sb = consts.tile([T, 1], F32)
            res_sb = consts.tile([1, G_LOC], F32)
            bsrow_sb = consts.tile([1, 6 * H], BF16)
            nc.sync.dma_start(bsrow_sb[:], bsrow_d[:])
            ones512_sb = consts.tile([1, UNIT], BF16)
            nc.vector.memset(ones512_sb[:], 1.0)

            # Per-group setup: A/Bb first-layer matrices (alpha0-prescaled via
            # the host-side weight scaling), mask sources, running max.
            a_sbs, bb_sbs, ctx0_sbs, bmax_sbs = [None] * G_LOC, \
                [None] * G_LOC, [None] * G_LOC, [None] * G_LOC

            def setup_group(g):
                # A = (ctx @ a0*W0_top).T and Bb = (ent @ a0*W0_bot).T + a0*b0
                # are precomputed on the host (0.1% of the FLOPs)
                a_sb = consts.tile([H, T], BF16, tag=f"a{g}")
                bb_sb = consts.tile([H, T], BF16, tag=f"bb{g}")
                ctx0_sb = consts.tile([T, 1], F32, tag=f"ctx0_{g}")
                nc.sync.dma_start(a_sb[:], aT_d[g])
                nc.sync.dma_start(bb_sb[:], bbT_d[g])
                nc.sync.dma_start(ctx0_sb[:], ctx0_d[g])
                bmax_sb = consts.tile([T, N_BLOCKS // G_LOC], F32,
                                      tag=f"bmax{g}")
                a_sbs[g], bb_sbs[g] = a_sb, bb_sb
                ctx0_sbs[g], bmax_sbs[g] = ctx0_sb, bmax_sb

            from concourse.bass import broadcast_tensor_aps

            def _z_add(engine, z_sb, g, j0, nj, c0):
                a_ap = a_sbs[g][:].rearrange("p (o i) -> p o i", o=1)
                b_ap = bb_sbs[g][:, j0:j0 + nj].rearrange("p (j o) -> p j o",
                                                          o=1)
                a_b, b_b = broadcast_tensor_aps(a_ap, b_ap)
                z_ap = z_sb[:, c0:c0 + nj * T].rearrange("p (j i) -> p j i",
                                                         j=nj)
                engine.tensor_tensor(z_ap, a_b, b_b, op=ALU.add)

            def build_z(b, fill=False):
                """Layer-0 z for block b (16 j's): z[:, jl*T+i] = A[:,i]+Bb[:,j].
                Broadcast-AP adds on the GpSimd engine; during pipeline fill
                the first piece goes to the (still idle) DVE and the rest is
                split so tanh1 can start before the whole block is done."""
                g = b // (N_BLOCKS // G_LOC)
                c = b % (N_BLOCKS // G_LOC)
                z_sb = zpool.tile([H, BLOCK], BF16, tag="z")
                j0 = c * NJ_BLOCK
                if fill:
                    _z_add(nc.vector, z_sb, g, j0, 4, 0)
                    for s in range(1, 4):
                        _z_add(nc.gpsimd, z_sb, g, j0 + 4 * s, 4, 4 * s * T)
                else:
                    _z_add(nc.gpsimd, z_sb, g, j0, NJ_BLOCK, 0)
                return z_sb

            z_tiles = {}
            h_cur = {}
            hp_cur = {}
            sc_tiles = {}

            def stage_t1_pair(e):
                """Unit-pair e's layer-0 activation on the z half-block
                [H,1024]: deg-5 DVE poly, or exact ACT tanh (scale undoes the
                alpha0 prescale) on some pairs to balance the engines."""
                b, s = e // 2, e % 2
                hp_sb = hppool.tile([H, 2 * UNIT], BF16, tag="hp")
                zsl = z_tiles[b][:, s * 2 * UNIT:(s + 1) * 2 * UNIT]
                if e in ACT_T1_PAIRS:
                    nc.scalar.activation(hp_sb[:], zsl, AF.Tanh,
                                         scale=1.0 / ALPHA[0])
                else:
                    nc.vector._custom_dve(op5, out=hp_sb[:], in0=zsl,
                                          s0=P5S[0][0], s1=P5S[0][1])
                hp_cur[e] = hp_sb
                if s == 1:
                    del z_tiles[b]

            def stage_mm(u, l):
                # layers 1-2: one 512-col matmul into a one-bank PSUM tile
                if l == 1:
                    ht = hp_cur[u // 2][:, (u % 2) * UNIT:(u % 2 + 1) * UNIT]
                else:
                    ht = h_cur[u][:]
                early = bias_zero and u < EARLY_ACT_UNITS
                slot = (5 + l) if early else l
                ps = mmps.tile([H, UNIT], F32, tag="mm")
                need_bias_mm = not bias_zero
                if need_bias_mm:
                    # fold the (prescaled) bias in on the PE -- the DVE poly
                    # has no bias slot
                    nc.tensor.matmul(ps[:], bsrow_sb[0:1, l * H:(l + 1) * H],
                                     ones512_sb[:], start=True, stop=False)
                nc.tensor.matmul(ps[:], ws_sb[:, slot * H:(slot + 1) * H], ht,
                                 start=not need_bias_mm, stop=True)
                h_cur[u] = ps

            def stage_act(u, l):
                # layers 1-2: deg-5 poly on the DVE; during fill the first
                # units go exact on the (otherwise idle) ACT instead
                ps = h_cur[u]
                h2_sb = hpool.tile([H, UNIT], BF16, tag="h")
                if bias_zero and u < EARLY_ACT_UNITS:
                    nc.scalar.activation(h2_sb[:], ps[:], AF.Tanh)
                else:
                    nc.vector._custom_dve(op5, out=h2_sb[:], in0=ps[:],
                                          s0=P5S[l][0], s1=P5S[l][1])
                h_cur[u] = h2_sb

            def stage_pair(e, l):
                # layers 3-5: two 512-col matmuls into one [H,1024] two-bank
                # PSUM tile, then ONE exact-tanh ACT op over the pair (the
                # ~180ns ACT instruction overhead amortizes at 1024 cols).
                # During drain only ACT work remains, so the last pairs run
                # the deg-5 poly on the (otherwise idle) DVE instead.
                tail = bias_zero and e in TAIL_P5_PAIRS
                slot = (l + 5) if tail else l
                ps = prps.tile([H, 2 * UNIT], F32, tag="pr")
                for s in range(2):
                    u = 2 * e + s
                    if l == 3:
                        ht = h_cur.pop(u)[:]
                    else:
                        ht = hp_cur[e][:, s * UNIT:(s + 1) * UNIT]
                    nc.tensor.matmul(ps[:, s * UNIT:(s + 1) * UNIT],
                                     ws_sb[:, slot * H:(slot + 1) * H], ht,
                                     start=True, stop=True)
                hp_sb = hppool.tile([H, 2 * UNIT], BF16, tag="hp")
                if tail:
                    nc.vector._custom_dve(op5, out=hp_sb[:], in0=ps[:],
                                          s0=P5S[l][0], s1=P5S[l][1])
                else:
                    nc.scalar.activation(hp_sb[:], ps[:], AF.Tanh,
                                         bias=bsT_sb[:, l:l + 1])
                hp_cur[e] = hp_sb

            def stage_score(u):
                # final layer, transposed: per j, score col [T(i), 1] via
                # stationary h-block x moving W_out
                b, s = u // U_PER_BLOCK, u % U_PER_BLOCK
                if s == 0:
                    sc_new = scps.tile([H, UNIT], F32, tag="sc")
                    sc_tiles[b] = sc_new
                sc_ps = sc_tiles[b]
                e, half = u // 2, u % 2
                ht = hp_cur[e]
                for jl in range(4):
                    o = half * UNIT + jl * T
                    nc.tensor.matmul(
                        sc_ps[:, s * 4 + jl:s * 4 + jl + 1],
                        ht[:, o:o + T],
                        wout_sb[:], start=True, stop=True)
                if half == 1:
                    del hp_cur[e]

            def stage_reduce(u):
                # max over the block's 16 j's into the group's per-block max
                # column (independent writes -- no serial running-max chain)
                b = u // U_PER_BLOCK
                nb = N_BLOCKS // G_LOC
                g, bl = b // nb, b % nb
                nc.vector.tensor_reduce(bmax_sbs[g][:, bl:bl + 1],
                                        sc_tiles.pop(b)[:, 0:NJ_BLOCK],
                                        axis=AX.X, op=ALU.max)

            def finalize_group(g):
                # mask = (ctx[:,0] != 0); out = sum_i(mask*(rmax+b_out))
                mask_sb = small.tile([T, 1], F32, tag="mask")
                nc.vector.tensor_scalar(mask_sb[:], ctx0_sbs[g][:], 0.0, None,
                                        op0=ALU.not_equal)
                rmax_sb = small.tile([T, 1], F32, tag="rmax")
                nc.vector.tensor_reduce(rmax_sb[:], bmax_sbs[g][:],
                                        axis=AX.X, op=ALU.max)
                rb_sb = small.tile([T, 1], F32, tag="rb")
                nc.vector.tensor_scalar_add(rb_sb[:], rmax_sb[:],
                                            bout_sb[:, 0:1])
                mm_sb = small.tile([T, 1], F32, tag="mmul")
                nc.vector.tensor_mul(mm_sb[:], rb_sb[:], mask_sb[:])
                # partition-axis sum via ones-matmul: [1,1] = mm.T @ ones
                sum_ps = scps.tile([H, UNIT], F32, tag="sc")
                nc.tensor.matmul(sum_ps[0:1, 0:1], mm_sb[:], ones_sb[:],
                                 start=True, stop=True)
                nc.vector.tensor_copy(res_sb[0:1, g:g + 1], sum_ps[0:1, 0:1])

            # group 0 first (the z(0) -> tanh1 chain gates pipeline fill),
            # then the weight DMAs (layer 1 isn't needed until position ~5)
            setup_group(0)
            nc.sync.dma_start(ws_sb[:, H:2 * H], ws_d[1])
            nc.sync.dma_start(bsT_sb[:], bsT_d[:])
            for l in range(2, 6):
                nc.sync.dma_start(ws_sb[:, l * H:(l + 1) * H], ws_d[l])
            for k in range(5):
                nc.sync.dma_start(ws_sb[:, (6 + k) * H:(7 + k) * H], wsx_d[k])
            nc.sync.dma_start(wout_sb[:], wout_d[:])
            nc.sync.dma_start(bout_sb[:], bout_d[:])
            nc.vector.memset(ones_sb[:], 1.0)

            # Software pipeline, one stage per position.  Layers 1-2 advance
            # per 512-pair unit u; layer 0 and layers 3-5 advance per
            # unit-pair e (positions 2e+d).  Stage delays interleave the two
            # granularities so each engine sees a near-constant load; the
            # emission order within a position respects same-position
            # producer/consumer hand-offs (L2act(2e+1) before the L3 pair
            # matmuls; the block reduce before the next block's score, which
            # recycles the single score PSUM buffer).
            D_T1 = 3          # tanh1 pair at 2e+3 (odd positions)
            D_MM1, D_ACT1 = 5, 6
            D_MM2, D_ACT2 = 7, 8
            D_L3 = 10         # pair mms+act at 2e+10 (even)
            D_L4 = 13         # 2e+13 (odd)
            D_L5 = 16         # 2e+16 (even)
            D_SC = 17         # score per unit at u+17
            D_RED = 18        # block reduce at (4b+3)+18
            NPOS = N_UNITS + D_RED + 1

            def live(u):
                return 0 <= u < N_UNITS

            for p in range(NPOS):
                # group setup trickles in ahead of its first unit
                for g in range(1, G_LOC):
                    if p == g * U_PER_GROUP - 10:
                        setup_group(g)
                # GpSimd: z block every 4 positions, 3 ahead of its tanh1
                if p % U_PER_BLOCK == 0 and p // U_PER_BLOCK < N_BLOCKS:
                    b = p // U_PER_BLOCK
                    z_new = build_z(b, fill=(b == 0))
                    z_tiles[b] = z_new
                if live(p - D_RED) and (p - D_RED) % U_PER_BLOCK == 3:
                    stage_reduce(p - D_RED)
                # PE, oldest first
                if live(p - D_SC):
                    stage_score(p - D_SC)
                if (p - D_L5) % 2 == 0 and live(p - D_L5):
                    stage_pair((p - D_L5) // 2, 5)
                if (p - D_L4) % 2 == 0 and live(p - D_L4):
                    stage_pair((p - D_L4) // 2, 4)
                # DVE acts for layers 1-2 (L2act(2e+1) feeds this position's
                # L3 pair matmuls)
                if live(p - D_ACT2):
                    stage_act(p - D_ACT2, 2)
                if live(p - D_ACT1):
                    stage_act(p - D_ACT1, 1)
                if (p - D_L3) % 2 == 0 and live(p - D_L3):
                    stage_pair((p - D_L3) // 2, 3)
                if live(p - D_MM2):
                    stage_mm(p - D_MM2, 2)
                if live(p - D_MM1):
                    stage_mm(p - D_MM1, 1)
                if (p - D_T1) % 2 == 0 and live(p - D_T1):
                    stage_t1_pair((p - D_T1) // 2)
                # per-group finalize as soon as its last reduce is in
                for g in range(G_LOC - 1):
                    if p == (g + 1) * U_PER_GROUP - 1 + D_RED + 1:
                        finalize_group(g)

            finalize_group(G_LOC - 1)

            nc.sync.dma_start(out_d[:], res_sb[:])

    nc.compile()
    return nc


def _get_nc(bias_zero):
    if bias_zero not in _cached_nc:
        _cached_nc[bias_zero] = _build_program(bias_zero)
    return _cached_nc[bias_zero]


def _bf16(a):
    return np.ascontiguousarray(a.astype(ml_dtypes.bfloat16))


def _prep_in_maps(input, Ws, bs, W_out, b_out):
    input = np.ascontiguousarray(np.asarray(input, dtype=np.float32))
    Ws = np.asarray(Ws, dtype=np.float32)
    bs = np.asarray(bs, dtype=np.float32)
    W_out = np.asarray(W_out, dtype=np.float32)
    b_out = np.asarray(b_out, dtype=np.float32)

    ctx = input[:, :, 0].reshape(G, T, D)
    ent = input[:, :, 1].reshape(G, T, D)
    ctx0 = np.ascontiguousarray(ctx[:, :, 0]).reshape(G, T, 1)  # fp32
    # prescale: alpha_l folds the deg-5 leading coefficient into the weights
    Ws_mod = Ws.copy()
    for l in range(3):
        Ws_mod[l] = ALPHA[l] * Ws[l]
    ws_bf = _bf16(Ws_mod)
    wsx = _bf16(np.stack([Ws[1], Ws[2]] +
                         [ALPHA[l] * Ws[l] for l in (3, 4, 5)]))
    # layer-0 split, computed host-side with the same bf16 rounding the PE
    # would apply: A = (ctx @ a0*W0_top).T, Bb = (ent @ a0*W0_bot).T + a0*b0
    w0t_f = _bf16(ALPHA[0] * Ws[0][0:D]).astype(np.float32)
    w0b_f = _bf16(ALPHA[0] * Ws[0][D:H]).astype(np.float32)
    ctx_f = _bf16(ctx).astype(np.float32)
    ent_f = _bf16(ent).astype(np.float32)
    aT = _bf16((ctx_f @ w0t_f).transpose(0, 2, 1))        # [G, H, T]
    bbT = _bf16((ent_f @ w0b_f).transpose(0, 2, 1)
                + (ALPHA[0] * bs[0]).reshape(1, H, 1))
    bsT = np.ascontiguousarray(bs.T).copy()               # [H, 6]
    bsT[:, 0] *= ALPHA[0]                                 # folded into Bb
    bs_row = bs.copy()
    bs_row[1] *= ALPHA[1]
    bs_row[2] *= ALPHA[2]
    bsrow = _bf16(bs_row.reshape(1, 6 * H))
    wout = _bf16(W_out)
    bout = np.broadcast_to(b_out.reshape(1, 1), (T, 1)).copy()

    in_maps = []
    for k in range(N_CORES):
        sl = slice(k * G_LOC, (k + 1) * G_LOC)
        in_maps.append({
            "aT": np.ascontiguousarray(aT[sl]),
            "bbT": np.ascontiguousarray(bbT[sl]),
            "ctx0": np.ascontiguousarray(ctx0[sl]),
            "Ws": ws_bf,
            "Wsx": wsx,
            "bsT": bsT,
            "bsrow": bsrow,
            "wout": wout,
            "bout": bout,
        })
    return in_maps


def run_traced(trace=False, **inputs):
    """Returns (output [G], exec_time_ns or None)."""
    nc = _get_nc(bias_zero=bool(np.all(np.asarray(inputs["bs"]) == 0)))
    in_maps = _prep_in_maps(**inputs)
    res = run_bass_kernel_spmd(nc, in_maps, list(range(N_CORES)), trace=trace)
    out = np.concatenate([res.results[k]["out"].reshape(G_LOC)
                          for k in range(N_CORES)])
    return out, res.exec_time_ns


def kernel(**inputs) -> np.ndarray:
    out, _ = run_traced(trace=False, **inputs)
    return out
